# revision 16
# baseline (speedup 1.0000x reference)
"""Trainium2 Bass kernel for ChunkedMultiHeadCardPassingLayer (B=4, T=4096, C=1024).

Sharding: 8 cores = B(4) x T-halves(2). Each core computes output rows
[g*2048, (g+1)*2048) of batch b through the full pipeline. The only
cross-core dependency is the chunk-carry running sum: the second T-half
needs the first half's total, exchanged with a tiny pairwise AllGather
([1, C] fp32 per core).

Per-core layout:
  - activations kept in [t, c] orientation (t on partitions) so the
    within-chunk cumsum is a strictly-triangular matmul and both
    LayerNorms reduce along the free axis
  - the carry broadcast-add rides a K=1 matmul accumulating into the
    cumsum PSUM tile (stationary = ones row, moving = the carry row)
  - cards are transposed to [d, t] per head-pair with PE transpose-mode
    for the head MLP; the transpose lands even heads on partitions 0-63
    and odd heads on 64-127, which ho2 (column-tiled matmuls) and the
    projection (contiguous channel-block stationaries) consume directly
  - matmuls run in bf16 (fp32 PSUM accumulation); LN stats are batched
    across chunks and use a bit-trick Newton rsqrt on the vector engine
    so the scalar engine's activation-table set only switches once
    (sigmoid -> gelu)
"""

import sys

sys.path.insert(0, "/opt/trn_rl_repo")

import numpy as np
import ml_dtypes

import concourse.bass as bass
import concourse.tile as tile
from concourse import bacc, mybir
from concourse.bass_utils import run_bass_kernel_spmd
from concourse.dve_ops import AFFINE_THEN_ADD, AFFINE_MUL_REDUCE

F32 = mybir.dt.float32
BF16 = mybir.dt.bfloat16
I32 = mybir.dt.int32
AL = mybir.AluOpType
AF = mybir.ActivationFunctionType
X = mybir.AxisListType.X
BFNP = ml_dtypes.bfloat16

B, T, C = 4, 4096, 1024
H, CS, D = 16, 128, 64
EPS = 1e-5
NCORES = 8
TL = T // 2          # rows per core
NCH = TL // CS       # chunks per core
RSQRT_MAGIC = 0x5F3759DF


def _newton_rsqrt(nc, pool, v, p, n, tag):
    """y = 1/sqrt(v) elementwise for v > 0, [p, n] fp32, vector engine only."""
    y = pool.tile([p, n], F32, name=f"nry_{tag}")
    ti = pool.tile([p, n], I32, name=f"nri_{tag}")
    nc.vector.tensor_scalar(ti[:], v.bitcast(I32), 1, None, op0=AL.logical_shift_right)
    nc.vector.tensor_scalar(ti[:], ti[:], -1, None, op0=AL.mult)
    nc.vector.tensor_scalar(y[:].bitcast(I32), ti[:], RSQRT_MAGIC, None, op0=AL.add)
    nh = pool.tile([p, n], F32, name=f"nrh_{tag}")
    nc.vector.tensor_scalar(nh[:], v, -0.5, None, op0=AL.mult)
    ysq = pool.tile([p, n], F32, name=f"nrq_{tag}")
    for _ in range(3):
        # y <- y * (1.5 + (-0.5 v) * y^2)
        nc.vector.tensor_tensor(ysq[:], y[:], y[:], op=AL.mult)
        nc.vector.tensor_tensor(ysq[:], ysq[:], nh[:], op=AL.mult)
        nc.vector.scalar_tensor_tensor(y[:], ysq[:], 1.5, y[:],
                                       op0=AL.add, op1=AL.mult)
    return y


def build_nc(flags):
    """flags: (mgb, projb, h1b, h2b, lng, carry_gb) nonzero-emission booleans."""
    f_mgb, f_projb, f_h1b, f_h2b, f_lng, f_cgb = flags
    nc = bacc.Bacc("TRN2", target_bir_lowering=False, debug=False, num_devices=NCORES)

    dram_in = lambda n, s, d: nc.dram_tensor(n, s, d, kind="ExternalInput").ap()
    xT = dram_in("xT", [C, TL], BF16)
    xres = dram_in("xres", [TL, C], F32)
    wmg = dram_in("wmg", [C, 2 * C], BF16)
    wproj = dram_in("wproj", [C, C], BF16)
    w1x = dram_in("w1x", [2 * D, 2 * D], BF16)
    w1z = dram_in("w1z", [2 * D, 2 * D], BF16)
    w2 = dram_in("w2", [2 * D, D], BF16)
    tri = dram_in("tri", [CS, CS], BF16)
    tri16 = dram_in("tri16", [NCH, NCH], BF16)
    ones16 = dram_in("ones16", [NCH, 1], BF16)
    ones1 = dram_in("ones1", [CS, CS], BF16)
    ejs = dram_in("ejs", [CS, NCH * NCH], BF16)
    ident = dram_in("ident", [CS, CS], BF16)
    tsel = dram_in("tsel", [1, 1], F32)
    if f_mgb:
        wmgb = dram_in("wmgb", [1, 2 * C], BF16)
    if f_projb:
        wprojb = dram_in("wprojb", [1, C], BF16)
    if f_h1b:
        w1b = dram_in("w1b", [1, 2 * D], BF16)
    if f_h2b:
        w2b = dram_in("w2b", [1, D], BF16)
    if f_h1b or f_h2b:
        onesN = dram_in("onesN", [1, 8 * CS], BF16)
    if f_lng:
        lngb = dram_in("lngb", [128, C], F32)
    if f_cgb:
        cgb = dram_in("cgb", [NCH, 2 * C], F32)
    out = nc.dram_tensor("out", [TL, C], F32, kind="ExternalOutput").ap()

    with tile.TileContext(nc) as tc:
        with tc.tile_pool(name="const", bufs=1) as cp, \
             tc.tile_pool(name="bigbf", bufs=22) as bb, \
             tc.tile_pool(name="stats", bufs=1) as stp, \
             tc.tile_pool(name="stream", bufs=3) as strm, \
             tc.tile_pool(name="dram", bufs=1, space="DRAM") as dram:

            # ---------- resident weights & constants ----------
            xT_t, wmg_t, wproj_t = [], [], []
            for c in range(8):
                t1 = cp.tile([128, TL], BF16, name=f"xTt_{c}")
                nc.sync.dma_start(t1[:], xT[c * 128:(c + 1) * 128, :])
                xT_t.append(t1)
                t2 = cp.tile([128, 2 * C], BF16, name=f"wmgt_{c}")
                nc.sync.dma_start(t2[:], wmg[c * 128:(c + 1) * 128, :])
                wmg_t.append(t2)
                t3 = cp.tile([128, C], BF16, name=f"wprojt_{c}")
                nc.sync.dma_start(t3[:], wproj[c * 128:(c + 1) * 128, :])
                wproj_t.append(t3)

            def load_const(name, src, shape):
                t = cp.tile(shape, BF16, name=name)
                nc.sync.dma_start(t[:], src[:])
                return t

            w1x_t = load_const("w1xt", w1x, [2 * D, 2 * D])
            w1z_t = load_const("w1zt", w1z, [2 * D, 2 * D])
            w2_t = load_const("w2t", w2, [2 * D, D])
            tri_t = load_const("trit", tri, [CS, CS])
            tri16_t = load_const("tri16t", tri16, [NCH, NCH])
            ones16_t = load_const("ones16t", ones16, [NCH, 1])
            ones1_t = load_const("ones1t", ones1, [CS, CS])
            ejs_t = load_const("ejst", ejs, [CS, NCH * NCH])
            ident_t = load_const("identt", ident, [CS, CS])
            tsel_t = cp.tile([1, 1], F32, name="tselt")
            nc.sync.dma_start(tsel_t[:], tsel[:])
            if f_mgb:
                wmgb_t = load_const("wmgbt", wmgb, [1, 2 * C])
            if f_projb:
                wprojb_t = load_const("wprojbt", wprojb, [1, C])
            if f_h1b:
                w1b_t = load_const("w1bt", w1b, [1, 2 * D])
            if f_h2b:
                w2b_t = load_const("w2bt", w2b, [1, D])
            if f_h1b or f_h2b:
                onesN_t = load_const("onesNt", onesN, [1, 8 * CS])
            if f_lng:
                lngb_t = cp.tile([128, C], F32, name="lngbt")
                nc.sync.dma_start(lngb_t[:], lngb[:])
            if f_cgb:
                cgb_t = cp.tile([NCH, 2 * C], F32, name="cgbt")
                nc.sync.dma_start(cgb_t[:], cgb[:])

            gm_t = [None] * NCH

            # ============ loop1: mark/gate -> gm -> chunk totals ============
            with tc.tile_pool(name="ps1", bufs=3, space="PSUM") as ps1, \
                 tc.tile_pool(name="pscs", bufs=1, space="PSUM") as pscs:
                csum = pscs.tile([NCH, C], F32, name="csum")
                for j in range(NCH):
                    pm = ps1.tile([128, C], F32, name="pm", tag="ps1t")
                    gt = ps1.tile([128, C], F32, name="gt", tag="ps1t")
                    for c in range(8):
                        st = xT_t[c][:, j * CS:(j + 1) * CS]
                        last = (c == 7) and not f_mgb
                        nc.tensor.matmul(pm[:, 0:512], st, wmg_t[c][:, 0:512],
                                         start=(c == 0), stop=last)
                        nc.tensor.matmul(pm[:, 512:1024], st, wmg_t[c][:, 512:1024],
                                         start=(c == 0), stop=last)
                        nc.tensor.matmul(gt[:, 0:512], st, wmg_t[c][:, 1024:1536],
                                         start=(c == 0), stop=last)
                        nc.tensor.matmul(gt[:, 512:1024], st, wmg_t[c][:, 1536:2048],
                                         start=(c == 0), stop=last)
                    if f_mgb:
                        nc.tensor.matmul(pm[:, 0:512], ones1_t[0:1, :], wmgb_t[0:1, 0:512],
                                         start=False, stop=True)
                        nc.tensor.matmul(pm[:, 512:1024], ones1_t[0:1, :],
                                         wmgb_t[0:1, 512:1024], start=False, stop=True)
                        nc.tensor.matmul(gt[:, 0:512], ones1_t[0:1, :],
                                         wmgb_t[0:1, 1024:1536], start=False, stop=True)
                        nc.tensor.matmul(gt[:, 512:1024], ones1_t[0:1, :],
                                         wmgb_t[0:1, 1536:2048], start=False, stop=True)
                    gts = bb.tile([128, C], BF16, name="gts", tag="gts", bufs=2)
                    nc.scalar.activation(gts[:], gt[:], AF.Sigmoid)
                    gm = bb.tile([128, C], BF16, name=f"gm_{j}", tag="big")
                    nc.vector.tensor_tensor(gm[:], pm[:], gts[:], op=AL.mult)
                    gm_t[j] = gm
                    ej = ejs_t[:, j * NCH:(j + 1) * NCH]
                    nc.tensor.matmul(csum[:, 0:512], ej, gm[:, 0:512],
                                     start=(j == 0), stop=(j == NCH - 1),
                                     skip_group_check=True)
                    nc.tensor.matmul(csum[:, 512:1024], ej, gm[:, 512:1024],
                                     start=(j == 0), stop=(j == NCH - 1),
                                     skip_group_check=True)
                csum_sb = stp.tile([NCH, C], BF16, name="csum_sb")
                nc.vector.tensor_copy(csum_sb[:], csum[:])

            # ============ mid: pairwise AllGather + carry LayerNorm ==========
            with tc.tile_pool(name="psm", bufs=1, space="PSUM") as psm:
                tot = psm.tile([1, C], F32, name="tot")
                nc.tensor.matmul(tot[:, 0:512], ones16_t[:], csum_sb[:, 0:512],
                                 start=True, stop=True)
                nc.tensor.matmul(tot[:, 512:1024], ones16_t[:], csum_sb[:, 512:1024],
                                 start=True, stop=True)
                tot_sb = stp.tile([1, C], F32, name="tot_sb")
                nc.vector.tensor_copy(tot_sb[:], tot[:])

                cc_in = dram.tile([1, C], F32, name="cc_in")
                cc_out = dram.tile([2, C], F32, name="cc_out")
                nc.sync.dma_start(cc_in[:], tot_sb[:])
                nc.gpsimd.collective_compute(
                    "AllGather", AL.bypass,
                    replica_groups=[[0, 1], [2, 3], [4, 5], [6, 7]],
                    ins=[cc_in.opt()], outs=[cc_out.opt()],
                )
                gath = stp.tile([2, C], F32, name="gath")
                nc.sync.dma_start(gath[:], cc_out[:])
                carry_in = stp.tile([1, C], BF16, name="carry_in")
                nc.vector.tensor_scalar(carry_in[:], gath[0:1, :], tsel_t[0:1, 0:1],
                                        None, op0=AL.mult)

                carries = psm.tile([NCH, C], F32, name="carries")
                nc.tensor.matmul(carries[:, 0:512], tri16_t[:], csum_sb[:, 0:512],
                                 start=True, stop=False)
                nc.tensor.matmul(carries[:, 512:1024], tri16_t[:],
                                 csum_sb[:, 512:1024], start=True, stop=False)
                nc.tensor.matmul(carries[:, 0:512], ones1_t[0:1, 0:NCH],
                                 carry_in[0:1, 0:512], start=False, stop=True)
                nc.tensor.matmul(carries[:, 512:1024], ones1_t[0:1, 0:NCH],
                                 carry_in[0:1, 512:1024], start=False, stop=True)

                cS1 = stp.tile([NCH, H], F32, name="cS1")
                nc.vector.reduce_sum(cS1[:], carries[:].rearrange("p (s k) -> p s k", s=H),
                                     axis=X)
                csq = stp.tile([NCH, C], F32, name="csq")
                nc.scalar.activation(csq[:], carries[:], AF.Square)
                cS2 = stp.tile([NCH, H], F32, name="cS2")
                nc.vector.reduce_sum(cS2[:], csq[:].rearrange("p (s k) -> p s k", s=H),
                                     axis=X)
                cnegm = stp.tile([NCH, H], F32, name="cnegm")
                nc.vector.tensor_scalar(cnegm[:], cS1[:], -1.0 / D, None, op0=AL.mult)
                cmsq = stp.tile([NCH, H], F32, name="cmsq")
                nc.vector.tensor_tensor(cmsq[:], cnegm[:], cnegm[:], op=AL.mult)
                nc.vector.tensor_scalar(cmsq[:], cmsq[:], -1.0, None, op0=AL.mult)
                cv = stp.tile([NCH, H], F32, name="cv")
                nc.vector._custom_dve(AFFINE_THEN_ADD, out=cv[:], in0=cS2[:],
                                      in1=cmsq[:], s0=1.0 / D, s1=EPS)
                cr = _newton_rsqrt(nc, stp, cv[:], NCH, H, "c")
                if f_cgb:
                    nrm32 = stp.tile([NCH, C], F32, name="nrm32")
                    for h in range(H):
                        sl = slice(h * D, (h + 1) * D)
                        nc.vector.tensor_scalar(nrm32[:, sl], carries[:, sl],
                                                cnegm[:, h:h + 1], cr[:, h:h + 1],
                                                op0=AL.add, op1=AL.mult)
                    nc.vector.tensor_tensor(nrm32[:], nrm32[:], cgb_t[:, 0:C],
                                            op=AL.mult)
                    nrm = stp.tile([NCH, C], BF16, name="nrm")
                    nc.vector.tensor_tensor(nrm[:], nrm32[:], cgb_t[:, C:2 * C],
                                            op=AL.add)
                else:
                    nrm = stp.tile([NCH, C], BF16, name="nrm")
                    for h in range(H):
                        sl = slice(h * D, (h + 1) * D)
                        nc.vector.tensor_scalar(nrm[:, sl], carries[:, sl],
                                                cnegm[:, h:h + 1], cr[:, h:h + 1],
                                                op0=AL.add, op1=AL.mult)
                # spread nrm rows onto 32-aligned partitions (matmul operand
                # base partitions must be 0/32/64/96): chunk j lives at
                # partition 32*(j//4), free offset (j%4)*C
                nrmf = stp.tile([128, 4 * C], BF16, name="nrmf")
                for grp in range(4):
                    nc.sync.dma_start(
                        nrmf[32 * grp:32 * grp + 1, :].rearrange(
                            "p (s k) -> p s k", s=4),
                        nrm[grp * 4:(grp + 1) * 4, :])

            # ============ loop2a: cumsum + card LN stats ============
            S1a = stp.tile([128, NCH * H], F32, name="S1a")
            S2a = stp.tile([128, NCH * H], F32, name="S2a")
            Qs_t = [None] * NCH
            ys_t = [None] * NCH
            S1p = stp.tile([128, NCH], F32, name="S1p")
            S2p = stp.tile([128, NCH], F32, name="S2p")
            with tc.tile_pool(name="ps2", bufs=3, space="PSUM") as ps2, \
                 tc.tile_pool(name="pzt", bufs=1, space="PSUM") as pzt:
                for j in range(NCH):
                    q = ps2.tile([128, C], F32, name="q", tag="ps2t")
                    gm = gm_t[j]
                    nc.tensor.matmul(q[:, 0:512], tri_t[:], gm[:, 0:512],
                                     start=True, stop=False)
                    nc.tensor.matmul(q[:, 512:1024], tri_t[:], gm[:, 512:1024],
                                     start=True, stop=False)
                    np_, no_ = 32 * (j // 4), (j % 4) * C
                    nc.tensor.matmul(q[:, 0:512], ones1_t[np_:np_ + 1, :],
                                     nrmf[np_:np_ + 1, no_:no_ + 512],
                                     start=False, stop=True,
                                     tile_position=(np_, 0))
                    nc.tensor.matmul(q[:, 512:1024], ones1_t[np_:np_ + 1, :],
                                     nrmf[np_:np_ + 1, no_ + 512:no_ + 1024],
                                     start=False, stop=True,
                                     tile_position=(np_, 0))
                    qs = bb.tile([128, C], BF16, name=f"qs_{j}", tag="big")
                    nc.scalar.activation(qs[:], q[:], AF.Copy)
                    Qs_t[j] = qs
                    nc.vector.reduce_sum(S1a[:, j * H:(j + 1) * H],
                                         qs[:].rearrange("p (s k) -> p s k", s=H),
                                         axis=X)
                    qsq = bb.tile([128, C], BF16, name="qsq", tag="qsq", bufs=2)
                    nc.vector.tensor_tensor(qsq[:], qs[:], qs[:], op=AL.mult)
                    nc.vector.reduce_sum(S2a[:, j * H:(j + 1) * H],
                                         qsq[:].rearrange("p (s k) -> p s k", s=H),
                                         axis=X)

                # batched card-LN stat combine
                negma = stp.tile([128, NCH * H], F32, name="negma")
                nc.vector.tensor_scalar(negma[:], S1a[:], -1.0 / D, None, op0=AL.mult)
                msqa = stp.tile([128, NCH * H], F32, name="msqa")
                nc.vector.tensor_tensor(msqa[:], negma[:], negma[:], op=AL.mult)
                nc.vector.tensor_scalar(msqa[:], msqa[:], -1.0, None, op0=AL.mult)
                va = stp.tile([128, NCH * H], F32, name="va")
                nc.vector._custom_dve(AFFINE_THEN_ADD, out=va[:], in0=S2a[:],
                                      in1=msqa[:], s0=1.0 / D, s1=EPS)
                ra = _newton_rsqrt(nc, stp, va[:], 128, NCH * H, "a")
                negmra = stp.tile([128, NCH * H], F32, name="negmra")
                nc.vector.tensor_tensor(negmra[:], negma[:], ra[:], op=AL.mult)

                # ============ loop2b+2c: normalize -> MLP -> proj ============
                sqd = None
                for j in range(NCH):
                    qs = Qs_t[j]
                    z = bb.tile([128, C], BF16, name="z", tag="z", bufs=2)
                    for h in range(H):
                        sl = slice(h * D, (h + 1) * D)
                        cidx = j * H + h
                        if h % 2 == 0:
                            nc.vector.tensor_scalar(
                                z[:, sl], qs[:, sl],
                                negma[:, cidx:cidx + 1], ra[:, cidx:cidx + 1],
                                op0=AL.add, op1=AL.mult)
                        else:
                            nc.scalar.activation(
                                z[:, sl], qs[:, sl], AF.Identity,
                                bias=negmra[:, cidx:cidx + 1],
                                scale=ra[:, cidx:cidx + 1])
                    zt = pzt.tile([128, 8 * CS], BF16, name="zt")
                    for q2 in range(8):
                        nc.tensor.matmul(zt[:, q2 * CS:(q2 + 1) * CS],
                                         z[:, q2 * 128:(q2 + 1) * 128], ident_t[:],
                                         is_transpose=True,
                                         start=(q2 == 0), stop=(q2 == 7),
                                         skip_group_check=True)
                    zts = bb.tile([128, 8 * CS], BF16, name="zts", tag="z", bufs=2)
                    nc.vector.tensor_copy(zts[:], zt[:])

                    # PSUM groups must start/stop on identical regions, so all
                    # ho1 matmuls run at 128-column granularity; batched per
                    # stationary (one w1x load + one w1z load per parity).
                    h1e = ps2.tile([128, 8 * CS], F32, name="h1e", tag="ps2t")
                    h1o = ps2.tile([128, 8 * CS], F32, name="h1o", tag="ps2t")
                    # start=True clears has_written for the whole 2 KiB bank:
                    # only the first matmul per bank carries it; later writes
                    # to untouched columns overwrite-and-mark automatically.
                    for par, dst in ((0, h1e), (1, h1o)):
                        for q2 in range(8):
                            # head h = 2*q2 + par lives in c-tile q2, half `par`
                            rhs = xT_t[q2][par * 64:par * 64 + 64,
                                           j * CS:(j + 1) * CS]
                            nc.tensor.matmul(dst[:, q2 * CS:(q2 + 1) * CS],
                                             w1x_t[par * 64:par * 64 + 64, :], rhs,
                                             start=(q2 % 4 == 0), stop=False,
                                             tile_position=(par * 64, 0),
                                             skip_group_check=True)
                        for half in range(2):
                            zsl = zts[par * 64:par * 64 + 64,
                                      half * 512:(half + 1) * 512]
                            nc.tensor.matmul(dst[:, half * 512:(half + 1) * 512],
                                             w1z_t[par * 64:par * 64 + 64, :], zsl,
                                             start=False, stop=not f_h1b,
                                             tile_position=(par * 64, 0),
                                             skip_group_check=True)
                        if f_h1b:
                            for half in range(2):
                                nc.tensor.matmul(dst[:, half * 512:(half + 1) * 512],
                                                 w1b_t[:],
                                                 onesN_t[0:1, half * 512:(half + 1) * 512],
                                                 start=False, stop=True,
                                                 tile_position=(0, 0),
                                                 skip_group_check=True)
                    h1se = bb.tile([128, 8 * CS], BF16, name="h1se", tag="z", bufs=2)
                    nc.scalar.activation(h1se[:], h1e[:], AF.Gelu)
                    h1so = bb.tile([128, 8 * CS], BF16, name="h1so", tag="z", bufs=2)
                    nc.scalar.activation(h1so[:], h1o[:], AF.Gelu)

                    hop = ps2.tile([128, 8 * CS], F32, name="hop", tag="ps2t")
                    for par, h1s in ((0, h1se), (1, h1so)):
                        tp = (0, 64 * par)
                        pr = slice(par * 64, par * 64 + 64)
                        last = not f_h2b
                        nc.tensor.matmul(hop[pr, 0:512], w2_t[:], h1s[:, 0:512],
                                         start=True, stop=last,
                                         tile_position=tp)
                        nc.tensor.matmul(hop[pr, 512:1024], w2_t[:], h1s[:, 512:1024],
                                         start=True, stop=last,
                                         tile_position=tp)
                    if f_h2b:
                        for par in (0, 1):
                            pr = slice(par * 64, par * 64 + 64)
                            nc.tensor.matmul(hop[pr, 0:512], w2b_t[:],
                                             onesN_t[0:1, 0:512], start=False,
                                             stop=False, tile_position=(0, 64 * par))
                            nc.tensor.matmul(hop[pr, 512:1024], w2b_t[:],
                                             onesN_t[0:1, 512:1024], start=False,
                                             stop=(par == 1), tile_position=(0, 64 * par))
                    hops = bb.tile([128, 8 * CS], BF16, name=f"hops_{j}", tag="big")
                    nc.vector.tensor_copy(hops[:], hop[:])

                    pj = ps2.tile([128, C], F32, name="pj", tag="ps2t")
                    for q2 in range(8):
                        st = hops[:, q2 * CS:(q2 + 1) * CS]
                        last = (q2 == 7) and not f_projb
                        nc.tensor.matmul(pj[:, 0:512], st, wproj_t[q2][:, 0:512],
                                         start=(q2 == 0), stop=last)
                        nc.tensor.matmul(pj[:, 512:1024], st, wproj_t[q2][:, 512:1024],
                                         start=(q2 == 0), stop=last)
                    if f_projb:
                        nc.tensor.matmul(pj[:, 0:512], ones1_t[0:1, :], wprojb_t[0:1, 0:512],
                                         start=False, stop=True)
                        nc.tensor.matmul(pj[:, 512:1024], ones1_t[0:1, :],
                                         wprojb_t[0:1, 512:1024], start=False, stop=True)
                    ys = bb.tile([128, C], BF16, name=f"ys_{j}", tag="big")
                    nc.scalar.activation(ys[:], pj[:], AF.Copy)
                    ys_t[j] = ys
                    nc.vector.reduce_sum(S1p[:, j:j + 1], ys[:], axis=X)
                    sqd = bb.tile([128, C], BF16, name="sqd", tag="qsq", bufs=2)
                    nc.scalar.activation(sqd[:], ys[:], AF.Square,
                                         accum_out=S2p[:, j:j + 1])

            # ============ batched proj-LN stats + loop2d: finish ============
            negmp = stp.tile([128, NCH], F32, name="negmp")
            nc.vector.tensor_scalar(negmp[:], S1p[:], -1.0 / C, None, op0=AL.mult)
            msqp = stp.tile([128, NCH], F32, name="msqp")
            nc.vector.tensor_tensor(msqp[:], negmp[:], negmp[:], op=AL.mult)
            nc.vector.tensor_scalar(msqp[:], msqp[:], -1.0, None, op0=AL.mult)
            vp = stp.tile([128, NCH], F32, name="vp")
            nc.vector._custom_dve(AFFINE_THEN_ADD, out=vp[:], in0=S2p[:],
                                  in1=msqp[:], s0=1.0 / C, s1=EPS)
            rp = _newton_rsqrt(nc, stp, vp[:], 128, NCH, "p")
            negmrp = stp.tile([128, NCH], F32, name="negmrp")
            nc.vector.tensor_tensor(negmrp[:], negmp[:], rp[:], op=AL.mult)

            for j in range(NCH):
                xr = strm.tile([128, C], F32, name="xr", tag="xr")
                nc.sync.dma_start(xr[:], xres[j * CS:(j + 1) * CS, :])
                ost = strm.tile([128, C], F32, name="ost", tag="ost")
                if f_lng:
                    t1 = strm.tile([128, C], F32, name="lnt", tag="lnt")
                    nc.vector.tensor_scalar(t1[:], ys_t[j][:], negmp[:, j:j + 1],
                                            rp[:, j:j + 1], op0=AL.add, op1=AL.mult)
                    nc.vector.tensor_tensor(t1[:], t1[:], lngb_t[:], op=AL.mult)
                    nc.vector.tensor_tensor(ost[:], t1[:], xr[:], op=AL.add)
                else:
                    nc.vector._custom_dve(AFFINE_THEN_ADD, out=ost[:], in0=ys_t[j][:],
                                          in1=xr[:], s0=rp[:, j:j + 1],
                                          s1=negmrp[:, j:j + 1])
                nc.sync.dma_start(out[j * CS:(j + 1) * CS, :], ost[:])

    nc.compile()
    return nc


_CACHE = {}
_LAST_RESULTS = [None]


def _to_bf(a):
    return np.ascontiguousarray(np.asarray(a, dtype=np.float32).astype(BFNP))


def prepare(x, mark_W, mark_b, gate_W, gate_b, carry_g, carry_b,
            card_g, card_b, ho1_W, ho1_b, ho2_W, ho2_b,
            proj_W, proj_b, ln_g, ln_b):
    x = np.asarray(x, dtype=np.float32)
    mark_W = np.asarray(mark_W, dtype=np.float32)
    mark_b = np.asarray(mark_b, dtype=np.float32)
    gate_W = np.asarray(gate_W, dtype=np.float32)
    gate_b = np.asarray(gate_b, dtype=np.float32)
    carry_g = np.asarray(carry_g, dtype=np.float32)
    carry_b = np.asarray(carry_b, dtype=np.float32)
    card_g = np.asarray(card_g, dtype=np.float32)
    card_b = np.asarray(card_b, dtype=np.float32)
    ho1_W = np.asarray(ho1_W, dtype=np.float32)
    ho1_b = np.asarray(ho1_b, dtype=np.float32)
    ho2_W = np.asarray(ho2_W, dtype=np.float32)
    ho2_b = np.asarray(ho2_b, dtype=np.float32)
    proj_W = np.asarray(proj_W, dtype=np.float32)
    proj_b = np.asarray(proj_b, dtype=np.float32)
    ln_g = np.asarray(ln_g, dtype=np.float32)
    ln_b = np.asarray(ln_b, dtype=np.float32)

    flags = (
        bool(np.any(mark_b) or np.any(gate_b)),
        bool(np.any(proj_b)),
        bool(np.any(ho1_b) or np.any(card_b)),
        bool(np.any(ho2_b)),
        bool(np.any(ln_g != 1.0)),
        bool(np.any(carry_g != 1.0) or np.any(carry_b)),
    )
    # ---- host-side fold + shard prep (exact fp32 math) ----
    # card LN gain folds into the cards half of ho1_W; card bias into ho1_b.
    w1 = ho1_W.copy()
    w1[D:2 * D, :] = w1[D:2 * D, :] * card_g[:, None]
    b1 = ho1_b + card_b @ ho1_W[D:2 * D, :]
    wmg_np = _to_bf(np.concatenate([mark_W, gate_W], axis=1))
    wproj_np = _to_bf(proj_W)
    w1x_np = _to_bf(np.vstack([w1[0:D, :], w1[0:D, :]]))
    w1z_np = _to_bf(np.vstack([w1[D:2 * D, :], w1[D:2 * D, :]]))
    w2_np = _to_bf(ho2_W)
    tri_np = _to_bf(np.triu(np.ones((CS, CS), np.float32), 1))
    tri16_np = _to_bf(np.triu(np.ones((NCH, NCH), np.float32), 1))
    ones16_np = _to_bf(np.ones((NCH, 1), np.float32))
    ones1_np = _to_bf(np.ones((CS, CS), np.float32))
    ejs_np = np.zeros((CS, NCH * NCH), np.float32)
    for j in range(NCH):
        ejs_np[:, j * NCH + j] = 1.0
    ejs_np = _to_bf(ejs_np)
    ident_np = _to_bf(np.eye(CS, dtype=np.float32))

    common = dict(wmg=wmg_np, wproj=wproj_np, w1x=w1x_np, w1z=w1z_np, w2=w2_np,
                  tri=tri_np, tri16=tri16_np, ones16=ones16_np, ones1=ones1_np,
                  ejs=ejs_np, ident=ident_np)
    if flags[0]:
        common["wmgb"] = _to_bf(np.concatenate([mark_b, gate_b])[None, :])
    if flags[1]:
        common["wprojb"] = _to_bf(proj_b[None, :])
    if flags[2]:
        common["w1b"] = _to_bf(b1[None, :])
    if flags[3]:
        common["w2b"] = _to_bf(ho2_b[None, :])
    if flags[2] or flags[3]:
        common["onesN"] = _to_bf(np.ones((1, 8 * CS), np.float32))
    if flags[4]:
        common["lngb"] = np.ascontiguousarray(
            np.broadcast_to(ln_g[None, :], (128, C)), dtype=np.float32)
    if flags[5]:
        cg = np.broadcast_to(np.tile(carry_g, H)[None, :], (NCH, C))
        cb = np.broadcast_to(np.tile(carry_b, H)[None, :], (NCH, C))
        common["cgb"] = np.ascontiguousarray(
            np.concatenate([cg, cb], axis=1), dtype=np.float32)

    in_maps = []
    for core in range(NCORES):
        b, g = core // 2, core % 2
        rows = slice(g * TL, (g + 1) * TL)
        m = dict(common)
        m["xT"] = np.ascontiguousarray(x[b, rows, :].T.astype(BFNP))
        m["xres"] = np.ascontiguousarray(x[b, rows, :] + ln_b[None, :])
        m["tsel"] = np.full((1, 1), float(g), np.float32)
        in_maps.append(m)
    return flags, in_maps


def assemble(results):
    out = np.empty((B, T, C), np.float32)
    for core in range(NCORES):
        b, g = core // 2, core % 2
        out[b, g * TL:(g + 1) * TL, :] = results[core]["out"]
    return out


def kernel(**inputs):
    flags, in_maps = prepare(**inputs)
    if flags not in _CACHE:
        _CACHE[flags] = build_nc(flags)
    nc = _CACHE[flags]
    res = run_bass_kernel_spmd(nc, in_maps, core_ids=list(range(NCORES)))
    _LAST_RESULTS[0] = res
    return assemble(res.results)


# revision 25
# speedup vs baseline: 102.9449x; 102.9449x over previous
"""Trainium2 Bass kernel for ChunkedMultiHeadCardPassingLayer (B=4, T=4096, C=1024).

Sharding: 8 cores = B(4) x T-halves(2). Each core computes output rows
[g*2048, (g+1)*2048) of batch b through the full pipeline. The only
cross-core dependency is the chunk-carry running sum: the second T-half
needs the first half's total, exchanged with a tiny pairwise AllGather
([1, C] fp32 per core).

Per-core layout:
  - activations kept in [t, c] orientation (t on partitions) so the
    within-chunk cumsum is a strictly-triangular matmul and both
    LayerNorms reduce along the free axis
  - the carry broadcast-add rides a K=1 matmul accumulating into the
    cumsum PSUM tile (stationary = ones row, moving = the carry row)
  - cards are transposed to [d, t] per head-pair with PE transpose-mode
    for the head MLP; the transpose lands even heads on partitions 0-63
    and odd heads on 64-127, which ho2 (column-tiled matmuls) and the
    projection (contiguous channel-block stationaries) consume directly
  - matmuls run in bf16 (fp32 PSUM accumulation); LN stats are batched
    across chunks and use a bit-trick Newton rsqrt on the vector engine
    so the scalar engine's activation-table set only switches once
    (sigmoid -> gelu)
"""

import sys

sys.path.insert(0, "/opt/trn_rl_repo")

import numpy as np
import ml_dtypes

import concourse.bass as bass
import concourse.tile as tile
from concourse import bacc, mybir
from concourse.bass_utils import run_bass_kernel_spmd
from concourse.dve_ops import AFFINE_THEN_ADD, AFFINE_MUL_REDUCE

F32 = mybir.dt.float32
BF16 = mybir.dt.bfloat16
I32 = mybir.dt.int32
AL = mybir.AluOpType
AF = mybir.ActivationFunctionType
X = mybir.AxisListType.X
BFNP = ml_dtypes.bfloat16

B, T, C = 4, 4096, 1024
H, CS, D = 16, 128, 64
EPS = 1e-5
NCORES = 8
TL = T // 2          # rows per core
NCH = TL // CS       # chunks per core
RSQRT_MAGIC = 0x5F3759DF


def _newton_rsqrt(nc, pool, v, p, n, tag):
    """y = 1/sqrt(v) elementwise for v > 0, [p, n] fp32, vector engine only."""
    y = pool.tile([p, n], F32, name=f"nry_{tag}")
    ti = pool.tile([p, n], I32, name=f"nri_{tag}")
    nc.vector.tensor_scalar(ti[:], v.bitcast(I32), 1, None, op0=AL.logical_shift_right)
    nc.vector.tensor_scalar(ti[:], ti[:], -1, None, op0=AL.mult)
    nc.vector.tensor_scalar(y[:].bitcast(I32), ti[:], RSQRT_MAGIC, None, op0=AL.add)
    nh = pool.tile([p, n], F32, name=f"nrh_{tag}")
    nc.vector.tensor_scalar(nh[:], v, -0.5, None, op0=AL.mult)
    ysq = pool.tile([p, n], F32, name=f"nrq_{tag}")
    for _ in range(3):
        # y <- y * (1.5 + (-0.5 v) * y^2)
        nc.vector.tensor_tensor(ysq[:], y[:], y[:], op=AL.mult)
        nc.vector.tensor_tensor(ysq[:], ysq[:], nh[:], op=AL.mult)
        nc.vector.scalar_tensor_tensor(y[:], ysq[:], 1.5, y[:],
                                       op0=AL.add, op1=AL.mult)
    return y


def build_nc(flags):
    """flags: (mgb, projb, h1b, h2b, lng, carry_gb) nonzero-emission booleans."""
    f_mgb, f_projb, f_h1b, f_h2b, f_lng, f_cgb = flags
    nc = bacc.Bacc("TRN2", target_bir_lowering=False, debug=False, num_devices=NCORES)

    dram_in = lambda n, s, d: nc.dram_tensor(n, s, d, kind="ExternalInput").ap()
    xT = dram_in("xT", [C, TL], BF16)
    xres = dram_in("xres", [TL, C], F32)
    wmg = dram_in("wmg", [C, 2 * C], BF16)
    wproj = dram_in("wproj", [C, C], BF16)
    w1x = dram_in("w1x", [2 * D, 2 * D], BF16)
    w1z = dram_in("w1z", [2 * D, 2 * D], BF16)
    w2 = dram_in("w2", [2 * D, D], BF16)
    tri = dram_in("tri", [CS, CS], BF16)
    tri16 = dram_in("tri16", [NCH, NCH], BF16)
    ones16 = dram_in("ones16", [NCH, 1], BF16)
    ones1 = dram_in("ones1", [CS, CS], BF16)
    ejs = dram_in("ejs", [CS, NCH * NCH], BF16)
    ident = dram_in("ident", [CS, CS], BF16)
    tsel = dram_in("tsel", [1, 1], F32)
    if f_mgb:
        wmgb = dram_in("wmgb", [1, 2 * C], BF16)
    if f_projb:
        wprojb = dram_in("wprojb", [1, C], BF16)
    if f_h1b:
        w1b = dram_in("w1b", [1, 2 * D], BF16)
    if f_h2b:
        w2b = dram_in("w2b", [1, D], BF16)
    if f_h1b or f_h2b:
        onesN = dram_in("onesN", [1, 8 * CS], BF16)
    if f_lng:
        lngb = dram_in("lngb", [128, C], F32)
    if f_cgb:
        cgb = dram_in("cgb", [NCH, 2 * C], F32)
    out = nc.dram_tensor("out", [TL, C], F32, kind="ExternalOutput").ap()

    with tile.TileContext(nc) as tc:
        with tc.tile_pool(name="const", bufs=1) as cp, \
             tc.tile_pool(name="bigbf", bufs=20) as bb, \
             tc.tile_pool(name="stats", bufs=1) as stp, \
             tc.tile_pool(name="stream", bufs=2) as strm, \
             tc.tile_pool(name="dram", bufs=1, space="DRAM") as dram:

            # ---------- resident weights & constants ----------
            xT_t, wmg_t, wproj_t = [], [], []
            for c in range(8):
                t1 = cp.tile([128, TL], BF16, name=f"xTt_{c}")
                nc.sync.dma_start(t1[:], xT[c * 128:(c + 1) * 128, :])
                xT_t.append(t1)
                t2 = cp.tile([128, 2 * C], BF16, name=f"wmgt_{c}")
                nc.sync.dma_start(t2[:], wmg[c * 128:(c + 1) * 128, :])
                wmg_t.append(t2)
                t3 = cp.tile([128, C], BF16, name=f"wprojt_{c}")
                nc.sync.dma_start(t3[:], wproj[c * 128:(c + 1) * 128, :])
                wproj_t.append(t3)

            def load_const(name, src, shape):
                t = cp.tile(shape, BF16, name=name)
                nc.sync.dma_start(t[:], src[:])
                return t

            w1x_t = load_const("w1xt", w1x, [2 * D, 2 * D])
            w1z_t = load_const("w1zt", w1z, [2 * D, 2 * D])
            w2_t = load_const("w2t", w2, [2 * D, D])
            tri_t = load_const("trit", tri, [CS, CS])
            tri16_t = load_const("tri16t", tri16, [NCH, NCH])
            ones16_t = load_const("ones16t", ones16, [NCH, 1])
            ones1_t = load_const("ones1t", ones1, [CS, CS])
            ejs_t = load_const("ejst", ejs, [CS, NCH * NCH])
            ident_t = load_const("identt", ident, [CS, CS])
            tsel_t = cp.tile([1, 1], F32, name="tselt")
            nc.sync.dma_start(tsel_t[:], tsel[:])
            if f_mgb:
                wmgb_t = load_const("wmgbt", wmgb, [1, 2 * C])
            if f_projb:
                wprojb_t = load_const("wprojbt", wprojb, [1, C])
            if f_h1b:
                w1b_t = load_const("w1bt", w1b, [1, 2 * D])
            if f_h2b:
                w2b_t = load_const("w2bt", w2b, [1, D])
            if f_h1b or f_h2b:
                onesN_t = load_const("onesNt", onesN, [1, 8 * CS])
            if f_lng:
                lngb_t = cp.tile([128, C], F32, name="lngbt")
                nc.sync.dma_start(lngb_t[:], lngb[:])
            if f_cgb:
                cgb_t = cp.tile([NCH, 2 * C], F32, name="cgbt")
                nc.sync.dma_start(cgb_t[:], cgb[:])

            gm_t = [None] * NCH

            # ============ loop1: mark/gate -> gm -> chunk totals ============
            with tc.tile_pool(name="ps1", bufs=3, space="PSUM") as ps1, \
                 tc.tile_pool(name="pscs", bufs=1, space="PSUM") as pscs:
                csum = pscs.tile([NCH, C], F32, name="csum")
                for j in range(NCH):
                    pm = ps1.tile([128, C], F32, name="pm", tag="ps1t")
                    gt = ps1.tile([128, C], F32, name="gt", tag="ps1t")
                    for c in range(8):
                        st = xT_t[c][:, j * CS:(j + 1) * CS]
                        last = (c == 7) and not f_mgb
                        nc.tensor.matmul(pm[:, 0:512], st, wmg_t[c][:, 0:512],
                                         start=(c == 0), stop=last)
                        nc.tensor.matmul(pm[:, 512:1024], st, wmg_t[c][:, 512:1024],
                                         start=(c == 0), stop=last)
                        nc.tensor.matmul(gt[:, 0:512], st, wmg_t[c][:, 1024:1536],
                                         start=(c == 0), stop=last)
                        nc.tensor.matmul(gt[:, 512:1024], st, wmg_t[c][:, 1536:2048],
                                         start=(c == 0), stop=last)
                    if f_mgb:
                        nc.tensor.matmul(pm[:, 0:512], ones1_t[0:1, :], wmgb_t[0:1, 0:512],
                                         start=False, stop=True)
                        nc.tensor.matmul(pm[:, 512:1024], ones1_t[0:1, :],
                                         wmgb_t[0:1, 512:1024], start=False, stop=True)
                        nc.tensor.matmul(gt[:, 0:512], ones1_t[0:1, :],
                                         wmgb_t[0:1, 1024:1536], start=False, stop=True)
                        nc.tensor.matmul(gt[:, 512:1024], ones1_t[0:1, :],
                                         wmgb_t[0:1, 1536:2048], start=False, stop=True)
                    gts = bb.tile([128, C], BF16, name="gts", tag="gts", bufs=2)
                    nc.scalar.activation(gts[:], gt[:], AF.Sigmoid)
                    gm = bb.tile([128, C], BF16, name=f"gm_{j}", tag="big")
                    nc.vector.tensor_tensor(gm[:], pm[:], gts[:], op=AL.mult)
                    gm_t[j] = gm
                    ej = ejs_t[:, j * NCH:(j + 1) * NCH]
                    nc.tensor.matmul(csum[:, 0:512], ej, gm[:, 0:512],
                                     start=(j == 0), stop=(j == NCH - 1),
                                     skip_group_check=True)
                    nc.tensor.matmul(csum[:, 512:1024], ej, gm[:, 512:1024],
                                     start=(j == 0), stop=(j == NCH - 1),
                                     skip_group_check=True)
                csum_sb = stp.tile([NCH, C], BF16, name="csum_sb")
                nc.vector.tensor_copy(csum_sb[:], csum[:])

            # ============ mid: pairwise AllGather + carry LayerNorm ==========
            with tc.tile_pool(name="psm", bufs=1, space="PSUM") as psm:
                tot = psm.tile([1, C], F32, name="tot")
                nc.tensor.matmul(tot[:, 0:512], ones16_t[:], csum_sb[:, 0:512],
                                 start=True, stop=True)
                nc.tensor.matmul(tot[:, 512:1024], ones16_t[:], csum_sb[:, 512:1024],
                                 start=True, stop=True)
                tot_sb = stp.tile([1, C], F32, name="tot_sb")
                nc.vector.tensor_copy(tot_sb[:], tot[:])

                cc_in = dram.tile([1, C], F32, name="cc_in")
                cc_out = dram.tile([2, C], F32, name="cc_out")
                nc.sync.dma_start(cc_in[:], tot_sb[:])
                nc.gpsimd.collective_compute(
                    "AllGather", AL.bypass,
                    replica_groups=[[0, 1], [2, 3], [4, 5], [6, 7]],
                    ins=[cc_in.opt()], outs=[cc_out.opt()],
                )
                gath = stp.tile([2, C], F32, name="gath")
                nc.sync.dma_start(gath[:], cc_out[:])
                carry_in = stp.tile([1, C], BF16, name="carry_in")
                nc.vector.tensor_scalar(carry_in[:], gath[0:1, :], tsel_t[0:1, 0:1],
                                        None, op0=AL.mult)

                carries = psm.tile([NCH, C], F32, name="carries")
                nc.tensor.matmul(carries[:, 0:512], tri16_t[:], csum_sb[:, 0:512],
                                 start=True, stop=False)
                nc.tensor.matmul(carries[:, 512:1024], tri16_t[:],
                                 csum_sb[:, 512:1024], start=True, stop=False)
                nc.tensor.matmul(carries[:, 0:512], ones1_t[0:1, 0:NCH],
                                 carry_in[0:1, 0:512], start=False, stop=True)
                nc.tensor.matmul(carries[:, 512:1024], ones1_t[0:1, 0:NCH],
                                 carry_in[0:1, 512:1024], start=False, stop=True)

                cS1 = stp.tile([NCH, H], F32, name="cS1")
                nc.vector.reduce_sum(cS1[:], carries[:].rearrange("p (s k) -> p s k", s=H),
                                     axis=X)
                csq = stp.tile([NCH, C], F32, name="csq")
                nc.scalar.activation(csq[:], carries[:], AF.Square)
                cS2 = stp.tile([NCH, H], F32, name="cS2")
                nc.vector.reduce_sum(cS2[:], csq[:].rearrange("p (s k) -> p s k", s=H),
                                     axis=X)
                cnegm = stp.tile([NCH, H], F32, name="cnegm")
                nc.vector.tensor_scalar(cnegm[:], cS1[:], -1.0 / D, None, op0=AL.mult)
                cmsq = stp.tile([NCH, H], F32, name="cmsq")
                nc.vector.tensor_tensor(cmsq[:], cnegm[:], cnegm[:], op=AL.mult)
                nc.vector.tensor_scalar(cmsq[:], cmsq[:], -1.0, None, op0=AL.mult)
                cv = stp.tile([NCH, H], F32, name="cv")
                nc.vector._custom_dve(AFFINE_THEN_ADD, out=cv[:], in0=cS2[:],
                                      in1=cmsq[:], s0=1.0 / D, s1=EPS)
                cr = _newton_rsqrt(nc, stp, cv[:], NCH, H, "c")
                if f_cgb:
                    nrm32 = stp.tile([NCH, C], F32, name="nrm32")
                    for h in range(H):
                        sl = slice(h * D, (h + 1) * D)
                        nc.vector.tensor_scalar(nrm32[:, sl], carries[:, sl],
                                                cnegm[:, h:h + 1], cr[:, h:h + 1],
                                                op0=AL.add, op1=AL.mult)
                    nc.vector.tensor_tensor(nrm32[:], nrm32[:], cgb_t[:, 0:C],
                                            op=AL.mult)
                    nrm = stp.tile([NCH, C], BF16, name="nrm")
                    nc.vector.tensor_tensor(nrm[:], nrm32[:], cgb_t[:, C:2 * C],
                                            op=AL.add)
                else:
                    nrm = stp.tile([NCH, C], BF16, name="nrm")
                    for h in range(H):
                        sl = slice(h * D, (h + 1) * D)
                        nc.vector.tensor_scalar(nrm[:, sl], carries[:, sl],
                                                cnegm[:, h:h + 1], cr[:, h:h + 1],
                                                op0=AL.add, op1=AL.mult)
                # spread nrm rows onto 32-aligned partitions (matmul operand
                # base partitions must be 0/32/64/96): chunk j lives at
                # partition 32*(j//4), free offset (j%4)*C
                nrmf = stp.tile([128, 4 * C], BF16, name="nrmf")
                for grp in range(4):
                    nc.sync.dma_start(
                        nrmf[32 * grp:32 * grp + 1, :].rearrange(
                            "p (s k) -> p s k", s=4),
                        nrm[grp * 4:(grp + 1) * 4, :])

            # ============ loop2a: cumsum + card LN stats ============
            S1a = stp.tile([128, NCH * H], F32, name="S1a")
            S2a = stp.tile([128, NCH * H], F32, name="S2a")
            Qs_t = [None] * NCH
            S1p = stp.tile([128, NCH], F32, name="S1p")
            S2p = stp.tile([128, NCH], F32, name="S2p")
            negmp = stp.tile([128, NCH], F32, name="negmp")
            msqp = stp.tile([128, NCH], F32, name="msqp")
            vp = stp.tile([128, NCH], F32, name="vp")
            negmrp = stp.tile([128, NCH], F32, name="negmrp")
            with tc.tile_pool(name="ps2", bufs=3, space="PSUM") as ps2, \
                 tc.tile_pool(name="pzt", bufs=1, space="PSUM") as pzt:
                for j in range(NCH):
                    q = ps2.tile([128, C], F32, name="q", tag="ps2t")
                    gm = gm_t[j]
                    nc.tensor.matmul(q[:, 0:512], tri_t[:], gm[:, 0:512],
                                     start=True, stop=False)
                    nc.tensor.matmul(q[:, 512:1024], tri_t[:], gm[:, 512:1024],
                                     start=True, stop=False)
                    np_, no_ = 32 * (j // 4), (j % 4) * C
                    nc.tensor.matmul(q[:, 0:512], ones1_t[np_:np_ + 1, :],
                                     nrmf[np_:np_ + 1, no_:no_ + 512],
                                     start=False, stop=True,
                                     tile_position=(np_, 0))
                    nc.tensor.matmul(q[:, 512:1024], ones1_t[np_:np_ + 1, :],
                                     nrmf[np_:np_ + 1, no_ + 512:no_ + 1024],
                                     start=False, stop=True,
                                     tile_position=(np_, 0))
                    qs = bb.tile([128, C], BF16, name=f"qs_{j}", tag="big")
                    nc.scalar.activation(qs[:], q[:], AF.Copy)
                    Qs_t[j] = qs
                    nc.vector.reduce_sum(S1a[:, j * H:(j + 1) * H],
                                         qs[:].rearrange("p (s k) -> p s k", s=H),
                                         axis=X)
                    qsq = bb.tile([128, C], BF16, name="qsq", tag="qsq", bufs=2)
                    nc.scalar.activation(qsq[:], qs[:], AF.Square)
                    nc.vector.reduce_sum(S2a[:, j * H:(j + 1) * H],
                                         qsq[:].rearrange("p (s k) -> p s k", s=H),
                                         axis=X)

                # batched card-LN stat combine
                negma = stp.tile([128, NCH * H], F32, name="negma")
                nc.vector.tensor_scalar(negma[:], S1a[:], -1.0 / D, None, op0=AL.mult)
                msqa = stp.tile([128, NCH * H], F32, name="msqa")
                nc.vector.tensor_tensor(msqa[:], negma[:], negma[:], op=AL.mult)
                nc.vector.tensor_scalar(msqa[:], msqa[:], -1.0, None, op0=AL.mult)
                va = stp.tile([128, NCH * H], F32, name="va")
                nc.vector._custom_dve(AFFINE_THEN_ADD, out=va[:], in0=S2a[:],
                                      in1=msqa[:], s0=1.0 / D, s1=EPS)
                ra = _newton_rsqrt(nc, stp, va[:], 128, NCH * H, "a")
                negmra = stp.tile([128, NCH * H], F32, name="negmra")
                nc.vector.tensor_tensor(negmra[:], negma[:], ra[:], op=AL.mult)

                # ============ loop2b+2c: normalize -> MLP -> proj ============
                sqd = None
                for j in range(NCH):
                    qs = Qs_t[j]
                    z = bb.tile([128, C], BF16, name="z", tag="z", bufs=6)
                    for h in range(H):
                        sl = slice(h * D, (h + 1) * D)
                        cidx = j * H + h
                        if h % 2 == 0:
                            nc.vector.tensor_scalar(
                                z[:, sl], qs[:, sl],
                                negma[:, cidx:cidx + 1], ra[:, cidx:cidx + 1],
                                op0=AL.add, op1=AL.mult)
                        else:
                            nc.scalar.activation(
                                z[:, sl], qs[:, sl], AF.Identity,
                                bias=negmra[:, cidx:cidx + 1],
                                scale=ra[:, cidx:cidx + 1])
                    zt = pzt.tile([128, 8 * CS], BF16, name="zt")
                    for q2 in range(8):
                        nc.tensor.matmul(zt[:, q2 * CS:(q2 + 1) * CS],
                                         z[:, q2 * 128:(q2 + 1) * 128], ident_t[:],
                                         is_transpose=True,
                                         start=(q2 == 0), stop=(q2 == 7),
                                         skip_group_check=True)
                    zts = bb.tile([128, 8 * CS], BF16, name="zts", tag="z", bufs=6)
                    nc.vector.tensor_copy(zts[:], zt[:])

                    # PSUM groups must start/stop on identical regions, so all
                    # ho1 matmuls run at 128-column granularity; batched per
                    # stationary (one w1x load + one w1z load per parity).
                    # start=True clears has_written for the whole 2 KiB bank:
                    # only the first matmul per bank carries it; later writes
                    # to untouched columns overwrite-and-mark automatically.
                    # Parities run serially (gelu evacuates parity 0's PSUM
                    # tile before parity 1's matmuls) to cut PSUM pressure.
                    h1s_par = []
                    for par in (0, 1):
                        dst = ps2.tile([128, 8 * CS], F32, name="h1", tag="ps2t")
                        for q2 in range(8):
                            # head h = 2*q2 + par lives in c-tile q2, half `par`
                            rhs = xT_t[q2][par * 64:par * 64 + 64,
                                           j * CS:(j + 1) * CS]
                            nc.tensor.matmul(dst[:, q2 * CS:(q2 + 1) * CS],
                                             w1x_t[par * 64:par * 64 + 64, :], rhs,
                                             start=(q2 % 4 == 0), stop=False,
                                             tile_position=(par * 64, 0),
                                             skip_group_check=True)
                        for half in range(2):
                            zsl = zts[par * 64:par * 64 + 64,
                                      half * 512:(half + 1) * 512]
                            nc.tensor.matmul(dst[:, half * 512:(half + 1) * 512],
                                             w1z_t[par * 64:par * 64 + 64, :], zsl,
                                             start=False, stop=not f_h1b,
                                             tile_position=(par * 64, 0),
                                             skip_group_check=True)
                        if f_h1b:
                            for half in range(2):
                                nc.tensor.matmul(dst[:, half * 512:(half + 1) * 512],
                                                 w1b_t[:],
                                                 onesN_t[0:1, half * 512:(half + 1) * 512],
                                                 start=False, stop=True,
                                                 tile_position=(0, 0),
                                                 skip_group_check=True)
                        h1s = bb.tile([128, 8 * CS], BF16, name="h1s", tag="z", bufs=6)
                        nc.scalar.activation(h1s[:], dst[:], AF.Gelu)
                        h1s_par.append(h1s)
                    h1se, h1so = h1s_par

                    hop = pzt.tile([128, 8 * CS], F32, name="hop", tag="zt")
                    for par, h1s in ((0, h1se), (1, h1so)):
                        tp = (0, 64 * par)
                        pr = slice(par * 64, par * 64 + 64)
                        last = not f_h2b
                        nc.tensor.matmul(hop[pr, 0:512], w2_t[:], h1s[:, 0:512],
                                         start=True, stop=last,
                                         tile_position=tp)
                        nc.tensor.matmul(hop[pr, 512:1024], w2_t[:], h1s[:, 512:1024],
                                         start=True, stop=last,
                                         tile_position=tp)
                    if f_h2b:
                        for par in (0, 1):
                            pr = slice(par * 64, par * 64 + 64)
                            nc.tensor.matmul(hop[pr, 0:512], w2b_t[:],
                                             onesN_t[0:1, 0:512], start=False,
                                             stop=False, tile_position=(0, 64 * par))
                            nc.tensor.matmul(hop[pr, 512:1024], w2b_t[:],
                                             onesN_t[0:1, 512:1024], start=False,
                                             stop=(par == 1), tile_position=(0, 64 * par))
                    hops = bb.tile([128, 8 * CS], BF16, name=f"hops_{j}", tag="big")
                    nc.vector.tensor_copy(hops[:], hop[:])

                    pj = ps2.tile([128, C], F32, name="pj", tag="ps2t")
                    for q2 in range(8):
                        st = hops[:, q2 * CS:(q2 + 1) * CS]
                        last = (q2 == 7) and not f_projb
                        nc.tensor.matmul(pj[:, 0:512], st, wproj_t[q2][:, 0:512],
                                         start=(q2 == 0), stop=last)
                        nc.tensor.matmul(pj[:, 512:1024], st, wproj_t[q2][:, 512:1024],
                                         start=(q2 == 0), stop=last)
                    if f_projb:
                        nc.tensor.matmul(pj[:, 0:512], ones1_t[0:1, :], wprojb_t[0:1, 0:512],
                                         start=False, stop=True)
                        nc.tensor.matmul(pj[:, 512:1024], ones1_t[0:1, :],
                                         wprojb_t[0:1, 512:1024], start=False, stop=True)
                    # proj-LN stats + normalize + residual, per chunk (keeps
                    # the whole loop2 region pipelined with no stats barrier)
                    ys = bb.tile([128, C], BF16, name="ys", tag="z", bufs=6)
                    nc.scalar.activation(ys[:], pj[:], AF.Copy)
                    nc.vector.reduce_sum(S1p[:, j:j + 1], ys[:], axis=X)
                    sqd = bb.tile([128, C], BF16, name="sqd", tag="qsq", bufs=2)
                    nc.scalar.activation(sqd[:], ys[:], AF.Square,
                                         accum_out=S2p[:, j:j + 1])
                    nc.vector.tensor_scalar(negmp[:, j:j + 1], S1p[:, j:j + 1],
                                            -1.0 / C, None, op0=AL.mult)
                    nc.vector.tensor_tensor(msqp[:, j:j + 1], negmp[:, j:j + 1],
                                            negmp[:, j:j + 1], op=AL.mult)
                    nc.vector.tensor_scalar(msqp[:, j:j + 1], msqp[:, j:j + 1],
                                            -1.0, None, op0=AL.mult)
                    nc.vector._custom_dve(AFFINE_THEN_ADD, out=vp[:, j:j + 1],
                                          in0=S2p[:, j:j + 1],
                                          in1=msqp[:, j:j + 1], s0=1.0 / C, s1=EPS)
                    rp = _newton_rsqrt(nc, stp, vp[:, j:j + 1], 128, 1, f"p{j}")
                    nc.vector.tensor_tensor(negmrp[:, j:j + 1], negmp[:, j:j + 1],
                                            rp[:], op=AL.mult)

                    xr = strm.tile([128, C], F32, name="xr", tag="xr")
                    nc.sync.dma_start(xr[:], xres[j * CS:(j + 1) * CS, :])
                    ost = strm.tile([128, C], F32, name="ost", tag="ost")
                    if f_lng:
                        t1 = strm.tile([128, C], F32, name="lnt", tag="lnt")
                        nc.vector.tensor_scalar(t1[:], ys[:], negmp[:, j:j + 1],
                                                rp[:], op0=AL.add, op1=AL.mult)
                        nc.vector.tensor_tensor(t1[:], t1[:], lngb_t[:], op=AL.mult)
                        nc.vector.tensor_tensor(ost[:], t1[:], xr[:], op=AL.add)
                    else:
                        nc.vector._custom_dve(AFFINE_THEN_ADD, out=ost[:], in0=ys[:],
                                              in1=xr[:], s0=rp[:],
                                              s1=negmrp[:, j:j + 1])
                    nc.sync.dma_start(out[j * CS:(j + 1) * CS, :], ost[:])

    nc.compile()
    return nc


_CACHE = {}
_LAST_RESULTS = [None]


def _to_bf(a):
    return np.ascontiguousarray(np.asarray(a, dtype=np.float32).astype(BFNP))


def prepare(x, mark_W, mark_b, gate_W, gate_b, carry_g, carry_b,
            card_g, card_b, ho1_W, ho1_b, ho2_W, ho2_b,
            proj_W, proj_b, ln_g, ln_b):
    x = np.asarray(x, dtype=np.float32)
    mark_W = np.asarray(mark_W, dtype=np.float32)
    mark_b = np.asarray(mark_b, dtype=np.float32)
    gate_W = np.asarray(gate_W, dtype=np.float32)
    gate_b = np.asarray(gate_b, dtype=np.float32)
    carry_g = np.asarray(carry_g, dtype=np.float32)
    carry_b = np.asarray(carry_b, dtype=np.float32)
    card_g = np.asarray(card_g, dtype=np.float32)
    card_b = np.asarray(card_b, dtype=np.float32)
    ho1_W = np.asarray(ho1_W, dtype=np.float32)
    ho1_b = np.asarray(ho1_b, dtype=np.float32)
    ho2_W = np.asarray(ho2_W, dtype=np.float32)
    ho2_b = np.asarray(ho2_b, dtype=np.float32)
    proj_W = np.asarray(proj_W, dtype=np.float32)
    proj_b = np.asarray(proj_b, dtype=np.float32)
    ln_g = np.asarray(ln_g, dtype=np.float32)
    ln_b = np.asarray(ln_b, dtype=np.float32)

    flags = (
        bool(np.any(mark_b) or np.any(gate_b)),
        bool(np.any(proj_b)),
        bool(np.any(ho1_b) or np.any(card_b)),
        bool(np.any(ho2_b)),
        bool(np.any(ln_g != 1.0)),
        bool(np.any(carry_g != 1.0) or np.any(carry_b)),
    )
    # ---- host-side fold + shard prep (exact fp32 math) ----
    # card LN gain folds into the cards half of ho1_W; card bias into ho1_b.
    w1 = ho1_W.copy()
    w1[D:2 * D, :] = w1[D:2 * D, :] * card_g[:, None]
    b1 = ho1_b + card_b @ ho1_W[D:2 * D, :]
    wmg_np = _to_bf(np.concatenate([mark_W, gate_W], axis=1))
    wproj_np = _to_bf(proj_W)
    w1x_np = _to_bf(np.vstack([w1[0:D, :], w1[0:D, :]]))
    w1z_np = _to_bf(np.vstack([w1[D:2 * D, :], w1[D:2 * D, :]]))
    w2_np = _to_bf(ho2_W)
    tri_np = _to_bf(np.triu(np.ones((CS, CS), np.float32), 1))
    tri16_np = _to_bf(np.triu(np.ones((NCH, NCH), np.float32), 1))
    ones16_np = _to_bf(np.ones((NCH, 1), np.float32))
    ones1_np = _to_bf(np.ones((CS, CS), np.float32))
    ejs_np = np.zeros((CS, NCH * NCH), np.float32)
    for j in range(NCH):
        ejs_np[:, j * NCH + j] = 1.0
    ejs_np = _to_bf(ejs_np)
    ident_np = _to_bf(np.eye(CS, dtype=np.float32))

    common = dict(wmg=wmg_np, wproj=wproj_np, w1x=w1x_np, w1z=w1z_np, w2=w2_np,
                  tri=tri_np, tri16=tri16_np, ones16=ones16_np, ones1=ones1_np,
                  ejs=ejs_np, ident=ident_np)
    if flags[0]:
        common["wmgb"] = _to_bf(np.concatenate([mark_b, gate_b])[None, :])
    if flags[1]:
        common["wprojb"] = _to_bf(proj_b[None, :])
    if flags[2]:
        common["w1b"] = _to_bf(b1[None, :])
    if flags[3]:
        common["w2b"] = _to_bf(ho2_b[None, :])
    if flags[2] or flags[3]:
        common["onesN"] = _to_bf(np.ones((1, 8 * CS), np.float32))
    if flags[4]:
        common["lngb"] = np.ascontiguousarray(
            np.broadcast_to(ln_g[None, :], (128, C)), dtype=np.float32)
    if flags[5]:
        cg = np.broadcast_to(np.tile(carry_g, H)[None, :], (NCH, C))
        cb = np.broadcast_to(np.tile(carry_b, H)[None, :], (NCH, C))
        common["cgb"] = np.ascontiguousarray(
            np.concatenate([cg, cb], axis=1), dtype=np.float32)

    in_maps = []
    for core in range(NCORES):
        b, g = core // 2, core % 2
        rows = slice(g * TL, (g + 1) * TL)
        m = dict(common)
        m["xT"] = np.ascontiguousarray(x[b, rows, :].T.astype(BFNP))
        m["xres"] = np.ascontiguousarray(x[b, rows, :] + ln_b[None, :])
        m["tsel"] = np.full((1, 1), float(g), np.float32)
        in_maps.append(m)
    return flags, in_maps


def assemble(results):
    out = np.empty((B, T, C), np.float32)
    for core in range(NCORES):
        b, g = core // 2, core % 2
        out[b, g * TL:(g + 1) * TL, :] = results[core]["out"]
    return out


def kernel(**inputs):
    flags, in_maps = prepare(**inputs)
    if flags not in _CACHE:
        _CACHE[flags] = build_nc(flags)
    nc = _CACHE[flags]
    res = run_bass_kernel_spmd(nc, in_maps, core_ids=list(range(NCORES)))
    _LAST_RESULTS[0] = res
    return assemble(res.results)


# revision 26
# speedup vs baseline: 103.1121x; 1.0016x over previous
"""Trainium2 Bass kernel for ChunkedMultiHeadCardPassingLayer (B=4, T=4096, C=1024).

Sharding: 8 cores = B(4) x T-halves(2). Each core computes output rows
[g*2048, (g+1)*2048) of batch b through the full pipeline. The only
cross-core dependency is the chunk-carry running sum: the second T-half
needs the first half's total, exchanged with a tiny pairwise AllGather
([1, C] fp32 per core).

Per-core layout:
  - activations kept in [t, c] orientation (t on partitions) so the
    within-chunk cumsum is a strictly-triangular matmul and both
    LayerNorms reduce along the free axis
  - the carry broadcast-add rides a K=1 matmul accumulating into the
    cumsum PSUM tile (stationary = ones row, moving = the carry row)
  - cards are transposed to [d, t] per head-pair with PE transpose-mode
    for the head MLP; the transpose lands even heads on partitions 0-63
    and odd heads on 64-127, which ho2 (column-tiled matmuls) and the
    projection (contiguous channel-block stationaries) consume directly
  - matmuls run in bf16 (fp32 PSUM accumulation); LN stats are batched
    across chunks and use a bit-trick Newton rsqrt on the vector engine
    so the scalar engine's activation-table set only switches once
    (sigmoid -> gelu)
"""

import sys

sys.path.insert(0, "/opt/trn_rl_repo")

import numpy as np
import ml_dtypes

import concourse.bass as bass
import concourse.tile as tile
from concourse import bacc, mybir
from concourse.bass_utils import run_bass_kernel_spmd
from concourse.dve_ops import AFFINE_THEN_ADD, AFFINE_MUL_REDUCE

F32 = mybir.dt.float32
BF16 = mybir.dt.bfloat16
I32 = mybir.dt.int32
AL = mybir.AluOpType
AF = mybir.ActivationFunctionType
X = mybir.AxisListType.X
BFNP = ml_dtypes.bfloat16

B, T, C = 4, 4096, 1024
H, CS, D = 16, 128, 64
EPS = 1e-5
NCORES = 8
TL = T // 2          # rows per core
NCH = TL // CS       # chunks per core
RSQRT_MAGIC = 0x5F3759DF


def _newton_rsqrt(nc, pool, v, p, n, tag):
    """y = 1/sqrt(v) elementwise for v > 0, [p, n] fp32, vector engine only."""
    y = pool.tile([p, n], F32, name=f"nry_{tag}")
    ti = pool.tile([p, n], I32, name=f"nri_{tag}")
    nc.vector.tensor_scalar(ti[:], v.bitcast(I32), 1, None, op0=AL.logical_shift_right)
    nc.vector.tensor_scalar(ti[:], ti[:], -1, None, op0=AL.mult)
    nc.vector.tensor_scalar(y[:].bitcast(I32), ti[:], RSQRT_MAGIC, None, op0=AL.add)
    nh = pool.tile([p, n], F32, name=f"nrh_{tag}")
    nc.vector.tensor_scalar(nh[:], v, -0.5, None, op0=AL.mult)
    ysq = pool.tile([p, n], F32, name=f"nrq_{tag}")
    for _ in range(3):
        # y <- y * (1.5 + (-0.5 v) * y^2)
        nc.vector.tensor_tensor(ysq[:], y[:], y[:], op=AL.mult)
        nc.vector.tensor_tensor(ysq[:], ysq[:], nh[:], op=AL.mult)
        nc.vector.scalar_tensor_tensor(y[:], ysq[:], 1.5, y[:],
                                       op0=AL.add, op1=AL.mult)
    return y


def build_nc(flags):
    """flags: (mgb, projb, h1b, h2b, lng, carry_gb) nonzero-emission booleans."""
    f_mgb, f_projb, f_h1b, f_h2b, f_lng, f_cgb = flags
    nc = bacc.Bacc("TRN2", target_bir_lowering=False, debug=False, num_devices=NCORES)

    dram_in = lambda n, s, d: nc.dram_tensor(n, s, d, kind="ExternalInput").ap()
    xT = dram_in("xT", [C, TL], BF16)
    xres = dram_in("xres", [TL, C], F32)
    wmg = dram_in("wmg", [C, 2 * C], BF16)
    wproj = dram_in("wproj", [C, C], BF16)
    w1x = dram_in("w1x", [2 * D, 2 * D], BF16)
    w1z = dram_in("w1z", [2 * D, 2 * D], BF16)
    w2 = dram_in("w2", [2 * D, D], BF16)
    tri = dram_in("tri", [CS, CS], BF16)
    tri16 = dram_in("tri16", [NCH, NCH], BF16)
    ones16 = dram_in("ones16", [NCH, 1], BF16)
    ones1 = dram_in("ones1", [CS, CS], BF16)
    ejs = dram_in("ejs", [CS, NCH * NCH], BF16)
    ident = dram_in("ident", [CS, CS], BF16)
    tsel = dram_in("tsel", [1, 1], F32)
    if f_mgb:
        wmgb = dram_in("wmgb", [1, 2 * C], BF16)
    if f_projb:
        wprojb = dram_in("wprojb", [1, C], BF16)
    if f_h1b:
        w1b = dram_in("w1b", [1, 2 * D], BF16)
    if f_h2b:
        w2b = dram_in("w2b", [1, D], BF16)
    if f_h1b or f_h2b:
        onesN = dram_in("onesN", [1, 8 * CS], BF16)
    if f_lng:
        lngb = dram_in("lngb", [128, C], F32)
    if f_cgb:
        cgb = dram_in("cgb", [NCH, 2 * C], F32)
    out = nc.dram_tensor("out", [TL, C], F32, kind="ExternalOutput").ap()

    with tile.TileContext(nc) as tc:
        with tc.tile_pool(name="const", bufs=1) as cp, \
             tc.tile_pool(name="bigbf", bufs=20) as bb, \
             tc.tile_pool(name="stats", bufs=1) as stp, \
             tc.tile_pool(name="stream", bufs=2) as strm, \
             tc.tile_pool(name="dram", bufs=1, space="DRAM") as dram:

            # ---------- resident weights & constants ----------
            xT_t, wmg_t, wproj_t = [], [], []
            for c in range(8):
                t1 = cp.tile([128, TL], BF16, name=f"xTt_{c}")
                nc.sync.dma_start(t1[:], xT[c * 128:(c + 1) * 128, :])
                xT_t.append(t1)
                t2 = cp.tile([128, 2 * C], BF16, name=f"wmgt_{c}")
                nc.sync.dma_start(t2[:], wmg[c * 128:(c + 1) * 128, :])
                wmg_t.append(t2)
                t3 = cp.tile([128, C], BF16, name=f"wprojt_{c}")
                nc.sync.dma_start(t3[:], wproj[c * 128:(c + 1) * 128, :])
                wproj_t.append(t3)

            def load_const(name, src, shape):
                t = cp.tile(shape, BF16, name=name)
                nc.sync.dma_start(t[:], src[:])
                return t

            w1x_t = load_const("w1xt", w1x, [2 * D, 2 * D])
            w1z_t = load_const("w1zt", w1z, [2 * D, 2 * D])
            w2_t = load_const("w2t", w2, [2 * D, D])
            tri_t = load_const("trit", tri, [CS, CS])
            tri16_t = load_const("tri16t", tri16, [NCH, NCH])
            ones16_t = load_const("ones16t", ones16, [NCH, 1])
            ones1_t = load_const("ones1t", ones1, [CS, CS])
            ejs_t = load_const("ejst", ejs, [CS, NCH * NCH])
            ident_t = load_const("identt", ident, [CS, CS])
            tsel_t = cp.tile([1, 1], F32, name="tselt")
            nc.sync.dma_start(tsel_t[:], tsel[:])
            if f_mgb:
                wmgb_t = load_const("wmgbt", wmgb, [1, 2 * C])
            if f_projb:
                wprojb_t = load_const("wprojbt", wprojb, [1, C])
            if f_h1b:
                w1b_t = load_const("w1bt", w1b, [1, 2 * D])
            if f_h2b:
                w2b_t = load_const("w2bt", w2b, [1, D])
            if f_h1b or f_h2b:
                onesN_t = load_const("onesNt", onesN, [1, 8 * CS])
            if f_lng:
                lngb_t = cp.tile([128, C], F32, name="lngbt")
                nc.sync.dma_start(lngb_t[:], lngb[:])
            if f_cgb:
                cgb_t = cp.tile([NCH, 2 * C], F32, name="cgbt")
                nc.sync.dma_start(cgb_t[:], cgb[:])

            gm_t = [None] * NCH

            # ============ loop1: mark/gate -> gm -> chunk totals ============
            with tc.tile_pool(name="ps1", bufs=3, space="PSUM") as ps1, \
                 tc.tile_pool(name="pscs", bufs=1, space="PSUM") as pscs:
                csum = pscs.tile([NCH, C], F32, name="csum")
                for j in range(NCH):
                    pm = ps1.tile([128, C], F32, name="pm", tag="ps1t")
                    gt = ps1.tile([128, C], F32, name="gt", tag="ps1t")
                    for c in range(8):
                        st = xT_t[c][:, j * CS:(j + 1) * CS]
                        last = (c == 7) and not f_mgb
                        nc.tensor.matmul(pm[:, 0:512], st, wmg_t[c][:, 0:512],
                                         start=(c == 0), stop=last)
                        nc.tensor.matmul(pm[:, 512:1024], st, wmg_t[c][:, 512:1024],
                                         start=(c == 0), stop=last)
                        nc.tensor.matmul(gt[:, 0:512], st, wmg_t[c][:, 1024:1536],
                                         start=(c == 0), stop=last)
                        nc.tensor.matmul(gt[:, 512:1024], st, wmg_t[c][:, 1536:2048],
                                         start=(c == 0), stop=last)
                    if f_mgb:
                        nc.tensor.matmul(pm[:, 0:512], ones1_t[0:1, :], wmgb_t[0:1, 0:512],
                                         start=False, stop=True)
                        nc.tensor.matmul(pm[:, 512:1024], ones1_t[0:1, :],
                                         wmgb_t[0:1, 512:1024], start=False, stop=True)
                        nc.tensor.matmul(gt[:, 0:512], ones1_t[0:1, :],
                                         wmgb_t[0:1, 1024:1536], start=False, stop=True)
                        nc.tensor.matmul(gt[:, 512:1024], ones1_t[0:1, :],
                                         wmgb_t[0:1, 1536:2048], start=False, stop=True)
                    gts = bb.tile([128, C], BF16, name="gts", tag="gts", bufs=2)
                    nc.scalar.activation(gts[:], gt[:], AF.Sigmoid)
                    gm = bb.tile([128, C], BF16, name=f"gm_{j}", tag="big")
                    nc.vector.tensor_tensor(gm[:], pm[:], gts[:], op=AL.mult)
                    gm_t[j] = gm
                    ej = ejs_t[:, j * NCH:(j + 1) * NCH]
                    nc.tensor.matmul(csum[:, 0:512], ej, gm[:, 0:512],
                                     start=(j == 0), stop=(j == NCH - 1),
                                     skip_group_check=True)
                    nc.tensor.matmul(csum[:, 512:1024], ej, gm[:, 512:1024],
                                     start=(j == 0), stop=(j == NCH - 1),
                                     skip_group_check=True)
                csum_sb = stp.tile([NCH, C], BF16, name="csum_sb")
                nc.vector.tensor_copy(csum_sb[:], csum[:])

            # ============ mid: pairwise AllGather + carry LayerNorm ==========
            with tc.tile_pool(name="psm", bufs=1, space="PSUM") as psm:
                tot = psm.tile([1, C], F32, name="tot")
                nc.tensor.matmul(tot[:, 0:512], ones16_t[:], csum_sb[:, 0:512],
                                 start=True, stop=True)
                nc.tensor.matmul(tot[:, 512:1024], ones16_t[:], csum_sb[:, 512:1024],
                                 start=True, stop=True)
                tot_sb = stp.tile([1, C], F32, name="tot_sb")
                nc.vector.tensor_copy(tot_sb[:], tot[:])

                cc_in = dram.tile([1, C], F32, name="cc_in")
                cc_out = dram.tile([2, C], F32, name="cc_out")
                nc.sync.dma_start(cc_in[:], tot_sb[:])
                nc.gpsimd.collective_compute(
                    "AllGather", AL.bypass,
                    replica_groups=[[0, 1], [2, 3], [4, 5], [6, 7]],
                    ins=[cc_in.opt()], outs=[cc_out.opt()],
                )
                gath = stp.tile([2, C], F32, name="gath")
                nc.sync.dma_start(gath[:], cc_out[:])
                carry_in = stp.tile([1, C], BF16, name="carry_in")
                nc.vector.tensor_scalar(carry_in[:], gath[0:1, :], tsel_t[0:1, 0:1],
                                        None, op0=AL.mult)

                carries = psm.tile([NCH, C], F32, name="carries")
                nc.tensor.matmul(carries[:, 0:512], tri16_t[:], csum_sb[:, 0:512],
                                 start=True, stop=False)
                nc.tensor.matmul(carries[:, 512:1024], tri16_t[:],
                                 csum_sb[:, 512:1024], start=True, stop=False)
                nc.tensor.matmul(carries[:, 0:512], ones1_t[0:1, 0:NCH],
                                 carry_in[0:1, 0:512], start=False, stop=True)
                nc.tensor.matmul(carries[:, 512:1024], ones1_t[0:1, 0:NCH],
                                 carry_in[0:1, 512:1024], start=False, stop=True)

                cS1 = stp.tile([NCH, H], F32, name="cS1")
                nc.vector.reduce_sum(cS1[:], carries[:].rearrange("p (s k) -> p s k", s=H),
                                     axis=X)
                csq = stp.tile([NCH, C], F32, name="csq")
                nc.scalar.activation(csq[:], carries[:], AF.Square)
                cS2 = stp.tile([NCH, H], F32, name="cS2")
                nc.vector.reduce_sum(cS2[:], csq[:].rearrange("p (s k) -> p s k", s=H),
                                     axis=X)
                cnegm = stp.tile([NCH, H], F32, name="cnegm")
                nc.vector.tensor_scalar(cnegm[:], cS1[:], -1.0 / D, None, op0=AL.mult)
                cmsq = stp.tile([NCH, H], F32, name="cmsq")
                nc.vector.tensor_tensor(cmsq[:], cnegm[:], cnegm[:], op=AL.mult)
                nc.vector.tensor_scalar(cmsq[:], cmsq[:], -1.0, None, op0=AL.mult)
                cv = stp.tile([NCH, H], F32, name="cv")
                nc.vector._custom_dve(AFFINE_THEN_ADD, out=cv[:], in0=cS2[:],
                                      in1=cmsq[:], s0=1.0 / D, s1=EPS)
                cr = _newton_rsqrt(nc, stp, cv[:], NCH, H, "c")
                if f_cgb:
                    nrm32 = stp.tile([NCH, C], F32, name="nrm32")
                    for h in range(H):
                        sl = slice(h * D, (h + 1) * D)
                        nc.vector.tensor_scalar(nrm32[:, sl], carries[:, sl],
                                                cnegm[:, h:h + 1], cr[:, h:h + 1],
                                                op0=AL.add, op1=AL.mult)
                    nc.vector.tensor_tensor(nrm32[:], nrm32[:], cgb_t[:, 0:C],
                                            op=AL.mult)
                    nrm = stp.tile([NCH, C], BF16, name="nrm")
                    nc.vector.tensor_tensor(nrm[:], nrm32[:], cgb_t[:, C:2 * C],
                                            op=AL.add)
                else:
                    nrm = stp.tile([NCH, C], BF16, name="nrm")
                    for h in range(H):
                        sl = slice(h * D, (h + 1) * D)
                        nc.vector.tensor_scalar(nrm[:, sl], carries[:, sl],
                                                cnegm[:, h:h + 1], cr[:, h:h + 1],
                                                op0=AL.add, op1=AL.mult)
                # spread nrm rows onto 32-aligned partitions (matmul operand
                # base partitions must be 0/32/64/96): chunk j lives at
                # partition 32*(j//4), free offset (j%4)*C
                nrmf = stp.tile([128, 4 * C], BF16, name="nrmf")
                for grp in range(4):
                    nc.sync.dma_start(
                        nrmf[32 * grp:32 * grp + 1, :].rearrange(
                            "p (s k) -> p s k", s=4),
                        nrm[grp * 4:(grp + 1) * 4, :])

            # ============ loop2a: cumsum + card LN stats ============
            S1a = stp.tile([128, NCH * H], F32, name="S1a")
            S2a = stp.tile([128, NCH * H], F32, name="S2a")
            Qs_t = [None] * NCH
            S1p = stp.tile([128, NCH], F32, name="S1p")
            S2p = stp.tile([128, NCH], F32, name="S2p")
            negmp = stp.tile([128, NCH], F32, name="negmp")
            msqp = stp.tile([128, NCH], F32, name="msqp")
            vp = stp.tile([128, NCH], F32, name="vp")
            negmrp = stp.tile([128, NCH], F32, name="negmrp")
            with tc.tile_pool(name="ps2", bufs=3, space="PSUM") as ps2, \
                 tc.tile_pool(name="pzt", bufs=1, space="PSUM") as pzt:
                for j in range(NCH):
                    q = ps2.tile([128, C], F32, name="q", tag="ps2t")
                    gm = gm_t[j]
                    nc.tensor.matmul(q[:, 0:512], tri_t[:], gm[:, 0:512],
                                     start=True, stop=False)
                    nc.tensor.matmul(q[:, 512:1024], tri_t[:], gm[:, 512:1024],
                                     start=True, stop=False)
                    np_, no_ = 32 * (j // 4), (j % 4) * C
                    nc.tensor.matmul(q[:, 0:512], ones1_t[np_:np_ + 1, :],
                                     nrmf[np_:np_ + 1, no_:no_ + 512],
                                     start=False, stop=True,
                                     tile_position=(np_, 0))
                    nc.tensor.matmul(q[:, 512:1024], ones1_t[np_:np_ + 1, :],
                                     nrmf[np_:np_ + 1, no_ + 512:no_ + 1024],
                                     start=False, stop=True,
                                     tile_position=(np_, 0))
                    qs = bb.tile([128, C], BF16, name=f"qs_{j}", tag="big")
                    nc.scalar.activation(qs[:], q[:], AF.Copy)
                    Qs_t[j] = qs
                    nc.vector.reduce_sum(S1a[:, j * H:(j + 1) * H],
                                         qs[:].rearrange("p (s k) -> p s k", s=H),
                                         axis=X)
                    qsq = bb.tile([128, C], BF16, name="qsq", tag="qsq", bufs=2)
                    nc.scalar.activation(qsq[:], qs[:], AF.Square)
                    nc.vector.reduce_sum(S2a[:, j * H:(j + 1) * H],
                                         qsq[:].rearrange("p (s k) -> p s k", s=H),
                                         axis=X)

                # card-LN stat combine, in two chunk-halves so the normalize
                # pipeline can start while the second half's stats are pending
                negma = stp.tile([128, NCH * H], F32, name="negma")
                msqa = stp.tile([128, NCH * H], F32, name="msqa")
                va = stp.tile([128, NCH * H], F32, name="va")
                ra = stp.tile([128, NCH * H], F32, name="ra")
                negmra = stp.tile([128, NCH * H], F32, name="negmra")
                for hf in range(2):
                    sl = slice(hf * NCH * H // 2, (hf + 1) * NCH * H // 2)
                    nc.vector.tensor_scalar(negma[:, sl], S1a[:, sl], -1.0 / D,
                                            None, op0=AL.mult)
                    nc.vector.tensor_tensor(msqa[:, sl], negma[:, sl],
                                            negma[:, sl], op=AL.mult)
                    nc.vector.tensor_scalar(msqa[:, sl], msqa[:, sl], -1.0,
                                            None, op0=AL.mult)
                    nc.vector._custom_dve(AFFINE_THEN_ADD, out=va[:, sl],
                                          in0=S2a[:, sl], in1=msqa[:, sl],
                                          s0=1.0 / D, s1=EPS)
                    rh = _newton_rsqrt(nc, stp, va[:, sl], 128, NCH * H // 2,
                                       f"a{hf}")
                    nc.vector.tensor_copy(ra[:, sl], rh[:])
                    nc.vector.tensor_tensor(negmra[:, sl], negma[:, sl],
                                            ra[:, sl], op=AL.mult)

                def bcast_d(t, j):
                    # [128, 16] stat cols for chunk j broadcast along d=64
                    ap = t[:, j * H:(j + 1) * H]
                    return bass.AP(ap.tensor, ap.offset, [ap.ap[0], [1, H], [0, D]])

                # ============ loop2b+2c: normalize -> MLP -> proj ============
                sqd = None
                for j in range(NCH):
                    qs = Qs_t[j]
                    z = bb.tile([128, C], BF16, name="z", tag="z", bufs=6)
                    q3 = qs[:].rearrange("p (s k) -> p s k", s=H)
                    z3 = z[:].rearrange("p (s k) -> p s k", s=H)
                    nc.vector.tensor_tensor(z3, q3, bcast_d(ra, j), op=AL.mult)
                    nc.vector.tensor_tensor(z3, z3, bcast_d(negmra, j), op=AL.add)
                    zt = pzt.tile([128, 8 * CS], BF16, name="zt")
                    for q2 in range(8):
                        nc.tensor.matmul(zt[:, q2 * CS:(q2 + 1) * CS],
                                         z[:, q2 * 128:(q2 + 1) * 128], ident_t[:],
                                         is_transpose=True,
                                         start=(q2 == 0), stop=(q2 == 7),
                                         skip_group_check=True)
                    zts = bb.tile([128, 8 * CS], BF16, name="zts", tag="z", bufs=6)
                    nc.vector.tensor_copy(zts[:], zt[:])

                    # PSUM groups must start/stop on identical regions, so all
                    # ho1 matmuls run at 128-column granularity; batched per
                    # stationary (one w1x load + one w1z load per parity).
                    # start=True clears has_written for the whole 2 KiB bank:
                    # only the first matmul per bank carries it; later writes
                    # to untouched columns overwrite-and-mark automatically.
                    # Parities run serially (gelu evacuates parity 0's PSUM
                    # tile before parity 1's matmuls) to cut PSUM pressure.
                    h1s_par = []
                    for par in (0, 1):
                        dst = ps2.tile([128, 8 * CS], F32, name="h1", tag="ps2t")
                        for q2 in range(8):
                            # head h = 2*q2 + par lives in c-tile q2, half `par`
                            rhs = xT_t[q2][par * 64:par * 64 + 64,
                                           j * CS:(j + 1) * CS]
                            nc.tensor.matmul(dst[:, q2 * CS:(q2 + 1) * CS],
                                             w1x_t[par * 64:par * 64 + 64, :], rhs,
                                             start=(q2 % 4 == 0), stop=False,
                                             tile_position=(par * 64, 0),
                                             skip_group_check=True)
                        for half in range(2):
                            zsl = zts[par * 64:par * 64 + 64,
                                      half * 512:(half + 1) * 512]
                            nc.tensor.matmul(dst[:, half * 512:(half + 1) * 512],
                                             w1z_t[par * 64:par * 64 + 64, :], zsl,
                                             start=False, stop=not f_h1b,
                                             tile_position=(par * 64, 0),
                                             skip_group_check=True)
                        if f_h1b:
                            for half in range(2):
                                nc.tensor.matmul(dst[:, half * 512:(half + 1) * 512],
                                                 w1b_t[:],
                                                 onesN_t[0:1, half * 512:(half + 1) * 512],
                                                 start=False, stop=True,
                                                 tile_position=(0, 0),
                                                 skip_group_check=True)
                        h1s = bb.tile([128, 8 * CS], BF16, name="h1s", tag="z", bufs=6)
                        nc.scalar.activation(h1s[:], dst[:], AF.Gelu)
                        h1s_par.append(h1s)
                    h1se, h1so = h1s_par

                    hop = pzt.tile([128, 8 * CS], F32, name="hop", tag="zt")
                    for par, h1s in ((0, h1se), (1, h1so)):
                        tp = (0, 64 * par)
                        pr = slice(par * 64, par * 64 + 64)
                        last = not f_h2b
                        nc.tensor.matmul(hop[pr, 0:512], w2_t[:], h1s[:, 0:512],
                                         start=True, stop=last,
                                         tile_position=tp)
                        nc.tensor.matmul(hop[pr, 512:1024], w2_t[:], h1s[:, 512:1024],
                                         start=True, stop=last,
                                         tile_position=tp)
                    if f_h2b:
                        for par in (0, 1):
                            pr = slice(par * 64, par * 64 + 64)
                            nc.tensor.matmul(hop[pr, 0:512], w2b_t[:],
                                             onesN_t[0:1, 0:512], start=False,
                                             stop=False, tile_position=(0, 64 * par))
                            nc.tensor.matmul(hop[pr, 512:1024], w2b_t[:],
                                             onesN_t[0:1, 512:1024], start=False,
                                             stop=(par == 1), tile_position=(0, 64 * par))
                    hops = bb.tile([128, 8 * CS], BF16, name=f"hops_{j}", tag="big")
                    nc.vector.tensor_copy(hops[:], hop[:])

                    pj = ps2.tile([128, C], F32, name="pj", tag="ps2t")
                    for q2 in range(8):
                        st = hops[:, q2 * CS:(q2 + 1) * CS]
                        last = (q2 == 7) and not f_projb
                        nc.tensor.matmul(pj[:, 0:512], st, wproj_t[q2][:, 0:512],
                                         start=(q2 == 0), stop=last)
                        nc.tensor.matmul(pj[:, 512:1024], st, wproj_t[q2][:, 512:1024],
                                         start=(q2 == 0), stop=last)
                    if f_projb:
                        nc.tensor.matmul(pj[:, 0:512], ones1_t[0:1, :], wprojb_t[0:1, 0:512],
                                         start=False, stop=True)
                        nc.tensor.matmul(pj[:, 512:1024], ones1_t[0:1, :],
                                         wprojb_t[0:1, 512:1024], start=False, stop=True)
                    # proj-LN stats + normalize + residual, per chunk (keeps
                    # the whole loop2 region pipelined with no stats barrier)
                    ys = bb.tile([128, C], BF16, name="ys", tag="z", bufs=6)
                    nc.scalar.activation(ys[:], pj[:], AF.Copy)
                    nc.vector.reduce_sum(S1p[:, j:j + 1], ys[:], axis=X)
                    sqd = bb.tile([128, C], BF16, name="sqd", tag="qsq", bufs=2)
                    nc.scalar.activation(sqd[:], ys[:], AF.Square,
                                         accum_out=S2p[:, j:j + 1])
                    nc.vector.tensor_scalar(negmp[:, j:j + 1], S1p[:, j:j + 1],
                                            -1.0 / C, None, op0=AL.mult)
                    nc.vector.tensor_tensor(msqp[:, j:j + 1], negmp[:, j:j + 1],
                                            negmp[:, j:j + 1], op=AL.mult)
                    nc.vector.tensor_scalar(msqp[:, j:j + 1], msqp[:, j:j + 1],
                                            -1.0, None, op0=AL.mult)
                    nc.vector._custom_dve(AFFINE_THEN_ADD, out=vp[:, j:j + 1],
                                          in0=S2p[:, j:j + 1],
                                          in1=msqp[:, j:j + 1], s0=1.0 / C, s1=EPS)
                    rp = _newton_rsqrt(nc, stp, vp[:, j:j + 1], 128, 1, f"p{j}")
                    nc.vector.tensor_tensor(negmrp[:, j:j + 1], negmp[:, j:j + 1],
                                            rp[:], op=AL.mult)

                    xr = strm.tile([128, C], F32, name="xr", tag="xr")
                    nc.sync.dma_start(xr[:], xres[j * CS:(j + 1) * CS, :])
                    ost = strm.tile([128, C], F32, name="ost", tag="ost")
                    if f_lng:
                        t1 = strm.tile([128, C], F32, name="lnt", tag="lnt")
                        nc.vector.tensor_scalar(t1[:], ys[:], negmp[:, j:j + 1],
                                                rp[:], op0=AL.add, op1=AL.mult)
                        nc.vector.tensor_tensor(t1[:], t1[:], lngb_t[:], op=AL.mult)
                        nc.vector.tensor_tensor(ost[:], t1[:], xr[:], op=AL.add)
                    else:
                        nc.vector._custom_dve(AFFINE_THEN_ADD, out=ost[:], in0=ys[:],
                                              in1=xr[:], s0=rp[:],
                                              s1=negmrp[:, j:j + 1])
                    nc.sync.dma_start(out[j * CS:(j + 1) * CS, :], ost[:])

    nc.compile()
    return nc


_CACHE = {}
_LAST_RESULTS = [None]


def _to_bf(a):
    return np.ascontiguousarray(np.asarray(a, dtype=np.float32).astype(BFNP))


def prepare(x, mark_W, mark_b, gate_W, gate_b, carry_g, carry_b,
            card_g, card_b, ho1_W, ho1_b, ho2_W, ho2_b,
            proj_W, proj_b, ln_g, ln_b):
    x = np.asarray(x, dtype=np.float32)
    mark_W = np.asarray(mark_W, dtype=np.float32)
    mark_b = np.asarray(mark_b, dtype=np.float32)
    gate_W = np.asarray(gate_W, dtype=np.float32)
    gate_b = np.asarray(gate_b, dtype=np.float32)
    carry_g = np.asarray(carry_g, dtype=np.float32)
    carry_b = np.asarray(carry_b, dtype=np.float32)
    card_g = np.asarray(card_g, dtype=np.float32)
    card_b = np.asarray(card_b, dtype=np.float32)
    ho1_W = np.asarray(ho1_W, dtype=np.float32)
    ho1_b = np.asarray(ho1_b, dtype=np.float32)
    ho2_W = np.asarray(ho2_W, dtype=np.float32)
    ho2_b = np.asarray(ho2_b, dtype=np.float32)
    proj_W = np.asarray(proj_W, dtype=np.float32)
    proj_b = np.asarray(proj_b, dtype=np.float32)
    ln_g = np.asarray(ln_g, dtype=np.float32)
    ln_b = np.asarray(ln_b, dtype=np.float32)

    flags = (
        bool(np.any(mark_b) or np.any(gate_b)),
        bool(np.any(proj_b)),
        bool(np.any(ho1_b) or np.any(card_b)),
        bool(np.any(ho2_b)),
        bool(np.any(ln_g != 1.0)),
        bool(np.any(carry_g != 1.0) or np.any(carry_b)),
    )
    # ---- host-side fold + shard prep (exact fp32 math) ----
    # card LN gain folds into the cards half of ho1_W; card bias into ho1_b.
    w1 = ho1_W.copy()
    w1[D:2 * D, :] = w1[D:2 * D, :] * card_g[:, None]
    b1 = ho1_b + card_b @ ho1_W[D:2 * D, :]
    wmg_np = _to_bf(np.concatenate([mark_W, gate_W], axis=1))
    wproj_np = _to_bf(proj_W)
    w1x_np = _to_bf(np.vstack([w1[0:D, :], w1[0:D, :]]))
    w1z_np = _to_bf(np.vstack([w1[D:2 * D, :], w1[D:2 * D, :]]))
    w2_np = _to_bf(ho2_W)
    tri_np = _to_bf(np.triu(np.ones((CS, CS), np.float32), 1))
    tri16_np = _to_bf(np.triu(np.ones((NCH, NCH), np.float32), 1))
    ones16_np = _to_bf(np.ones((NCH, 1), np.float32))
    ones1_np = _to_bf(np.ones((CS, CS), np.float32))
    ejs_np = np.zeros((CS, NCH * NCH), np.float32)
    for j in range(NCH):
        ejs_np[:, j * NCH + j] = 1.0
    ejs_np = _to_bf(ejs_np)
    ident_np = _to_bf(np.eye(CS, dtype=np.float32))

    common = dict(wmg=wmg_np, wproj=wproj_np, w1x=w1x_np, w1z=w1z_np, w2=w2_np,
                  tri=tri_np, tri16=tri16_np, ones16=ones16_np, ones1=ones1_np,
                  ejs=ejs_np, ident=ident_np)
    if flags[0]:
        common["wmgb"] = _to_bf(np.concatenate([mark_b, gate_b])[None, :])
    if flags[1]:
        common["wprojb"] = _to_bf(proj_b[None, :])
    if flags[2]:
        common["w1b"] = _to_bf(b1[None, :])
    if flags[3]:
        common["w2b"] = _to_bf(ho2_b[None, :])
    if flags[2] or flags[3]:
        common["onesN"] = _to_bf(np.ones((1, 8 * CS), np.float32))
    if flags[4]:
        common["lngb"] = np.ascontiguousarray(
            np.broadcast_to(ln_g[None, :], (128, C)), dtype=np.float32)
    if flags[5]:
        cg = np.broadcast_to(np.tile(carry_g, H)[None, :], (NCH, C))
        cb = np.broadcast_to(np.tile(carry_b, H)[None, :], (NCH, C))
        common["cgb"] = np.ascontiguousarray(
            np.concatenate([cg, cb], axis=1), dtype=np.float32)

    in_maps = []
    for core in range(NCORES):
        b, g = core // 2, core % 2
        rows = slice(g * TL, (g + 1) * TL)
        m = dict(common)
        m["xT"] = np.ascontiguousarray(x[b, rows, :].T.astype(BFNP))
        m["xres"] = np.ascontiguousarray(x[b, rows, :] + ln_b[None, :])
        m["tsel"] = np.full((1, 1), float(g), np.float32)
        in_maps.append(m)
    return flags, in_maps


def assemble(results):
    out = np.empty((B, T, C), np.float32)
    for core in range(NCORES):
        b, g = core // 2, core % 2
        out[b, g * TL:(g + 1) * TL, :] = results[core]["out"]
    return out


def kernel(**inputs):
    flags, in_maps = prepare(**inputs)
    if flags not in _CACHE:
        _CACHE[flags] = build_nc(flags)
    nc = _CACHE[flags]
    res = run_bass_kernel_spmd(nc, in_maps, core_ids=list(range(NCORES)))
    _LAST_RESULTS[0] = res
    return assemble(res.results)
